# revision 45
# baseline (speedup 1.0000x reference)
"""AttentionPooling (segment mean -> att = <x_i, coarse[batch_i]> -> weighted
segment mean -> Linear) on 8 Trainium2 NeuronCores.

Strategy
--------
`batch` is sorted and host-visible inside kernel(), so ALL index structure is
resolved on the host:

* The 8192 segments are bin-packed (LPT) into 512 groups of exactly 16
  segments, each group padded to 3968 rows (31 sub-tiles of 128 rows; max
  packed load is 3910 for this input).  Rows are permuted so every group is
  contiguous; pad rows are zero.  Each core owns 64 groups -> perfectly
  uniform SPMD program, no collectives (a segment never straddles cores).
* x is shipped twice: row-major bf16 `xg` (per-group partition-contiguous
  ~8 KB DMA lines) and transposed fp8e4m3 `xt` (att pass only; ct stays
  bf16 -- mixed-dtype matmul, rel err 1.2e-2 vs the 2e-2 gate).  DMAs are
  batched two groups per instruction and alternate between the SP and
  Activation HWDGE rings; output windows go via the gpsimd SWDGE queue so
  the two big rings only carry x.
* Per group g (P = one-hot row->slot mask, built by one DVE op):
    pass A:  psAT[d, slot] += xg_st^T @ P_st      (x stationary, N=16)
             ct = psAT * (1/count)                 [d, 16] bf16, no transpose
    pass B1: psAq[row, slot_st] = xt_st^T @ ct     (fp8 x^T stationary)
             Q = psAq * P    (DVE multiply, 4 chunks per group)
    pass B2: psPool[slot, d] += Q_st^T @ xg_st
             pooledT[d, slot] = transpose(psPool) * (1/count)
    final :  out[seg, :] = pooledT^T @ W^T + b, emitted per 128-segment
             window as soon as its 8 groups are pooled
"""

import os

import numpy as np
import ml_dtypes

import concourse.bass as bass
import concourse.mybir as mybir
import concourse.tile as tile
from concourse.bass_utils import run_bass_kernel_spmd
from concourse.vector_clock import ScopedClock

BF16 = mybir.dt.bfloat16
F8 = mybir.dt.float8e4
F32 = mybir.dt.float32
NP_F8 = ml_dtypes.float8_e4m3

N_CORES = 8
B_SEGS = 8192
D = 128
G = 16                  # segments (slots) per group
ST = 31                 # 128-row sub-tiles per group
GROUP_ROWS = ST * 128   # 3968
N_GROUPS = B_SEGS // G  # 512
GROUPS_PER_CORE = N_GROUPS // N_CORES  # 64
CORE_ROWS = GROUPS_PER_CORE * GROUP_ROWS  # 262144

LAST_RESULT = None  # BassKernelResults of the most recent run (for test.py)

_PATCHED = False


def _patch_tile_tail():
    """The walrus build in this container only lowers ONE sync-wait per
    instruction.  Tile routinely emits multi-wait instructions, so (a) split
    every scheduled instruction's extra waits onto injected same-engine NOPs
    (engines execute their stream in order, so a wait on a preceding NOP is
    equivalent), and (b) do the same for the TileContext exit drain."""
    global _PATCHED
    if _PATCHED:
        return
    _PATCHED = True

    orig_lower = tile.TileContext._lower_ordered_insts

    def _lower_ordered_insts(self, ordered):
        nid = [0]
        for bb_name, insts in ordered.items():
            new = []
            for inst in insts:
                si = inst.sync_info
                if si is not None and si.on_wait and len(si.on_wait) > 1:
                    waits = list(si.on_wait)
                    for w in waits[:-1]:
                        nid[0] += 1
                        nop = mybir.InstNoOp(
                            name=f"splitw-{nid[0]}-{inst.name}",
                            engine=inst.engine,
                            sync_info=mybir.SyncInfo(on_wait=[w], on_update=[]),
                            bass_nofuse=True,
                        )
                        new.append(nop)
                    si.on_wait = [waits[-1]]
                new.append(inst)
            ordered[bb_name] = new
        return orig_lower(self, ordered)

    tile.TileContext._lower_ordered_insts = _lower_ordered_insts

    def _drain_and_barrier(self, tick_clock, wait_clock):
        nc = self.nc
        probe = nc.sync.nop(nofuse=True, hint="tail_wait0")
        wait_clock.add_sem_waits(
            probe.ins, ScopedClock({None: tick_clock.global_clock})
        )
        si = probe.ins.sync_info
        waits = list(si.on_wait or []) if si is not None else []
        if len(waits) > 1:
            si.on_wait = waits[:1]
            for w in waits[1:]:
                n2 = nc.sync.nop(nofuse=True, hint="tail_wait")
                n2.ins.sync_info = mybir.SyncInfo(on_wait=[w], on_update=[])
        nc.sync.drain()
        nc.all_engine_barrier()
        popped = nc._tile_sem_poison_stack.pop()
        assert popped is self._sem_poison
        nc.clear_and_free_semaphores(list(self.sems.allocated().values()))
        nc.all_engine_barrier()

    tile.TileContext._drain_and_barrier = _drain_and_barrier


# --------------------------------------------------------------------------
# host-side packing
# --------------------------------------------------------------------------

def _pack_segments(counts):
    """Assign each segment to a (group, slot).  512 groups x 16 slots, rows
    per group <= GROUP_ROWS.  Balanced LPT dealing: 16 rounds; each round
    hands the next 512 largest segments to the currently lightest groups."""
    order = np.argsort(-counts, kind="stable")
    loads = np.zeros(N_GROUPS, dtype=np.int64)
    seg_ids = np.empty((N_GROUPS, G), dtype=np.int64)
    for r in range(G):
        chunk = order[r * N_GROUPS:(r + 1) * N_GROUPS]
        grp_order = np.argsort(loads, kind="stable")
        seg_ids[grp_order, r] = chunk
        loads[grp_order] += counts[chunk]
    assert loads.max() <= GROUP_ROWS, (
        f"group overflow: {loads.max()} > {GROUP_ROWS}"
    )
    return seg_ids  # [512, 16] segment id per (group, slot)


def _host_prepare(x, batch, W, b):
    counts = np.bincount(batch, minlength=B_SEGS).astype(np.int64)
    seg_start = np.concatenate([[0], np.cumsum(counts)[:-1]])
    seg_ids = _pack_segments(counts)                       # [512, 16]

    flat_segs = seg_ids.reshape(-1)                        # packed order
    flat_counts = counts[flat_segs]
    # destination start of each packed segment
    within = flat_counts.reshape(N_GROUPS, G)
    offs = np.cumsum(within, axis=1) - within              # [512, 16]
    dest_start = (np.arange(N_GROUPS)[:, None] * GROUP_ROWS + offs).reshape(-1)
    src_start = seg_start[flat_segs]

    total = int(flat_counts.sum())
    assert total == x.shape[0]
    rag = np.arange(total, dtype=np.int64) - np.repeat(
        np.cumsum(flat_counts) - flat_counts, flat_counts
    )
    valid_dest = np.repeat(dest_start, flat_counts) + rag
    valid_src = np.repeat(src_start, flat_counts) + rag

    n_pad = N_GROUPS * GROUP_ROWS
    x_bf = x.astype(ml_dtypes.bfloat16)
    x_pad = np.zeros((n_pad, D), dtype=ml_dtypes.bfloat16)
    x_pad[valid_dest] = x_bf[valid_src]
    # per-group partition-contiguous layout: [g*128+p, st*D+d]
    xg_grp = np.ascontiguousarray(
        x_pad.reshape(N_GROUPS, ST, 128, D).transpose(0, 2, 1, 3)
        .reshape(N_GROUPS * 128, ST * D)
    )

    x_f8 = x.astype(NP_F8)
    x8_pad = np.zeros((n_pad, D), dtype=NP_F8)
    x8_pad[valid_dest] = x_f8[valid_src]
    xt_pad = np.ascontiguousarray(x8_pad.T)                # [128, n_pad] fp8

    slotvec = np.zeros(n_pad, dtype=np.float32)
    slot_of_seg = np.repeat(
        np.tile(np.arange(G, dtype=np.float32), N_GROUPS), flat_counts
    )
    slotvec[valid_dest] = slot_of_seg

    invc = (1.0 / np.maximum(counts, 1)).astype(np.float32)
    invc_packed = invc[flat_segs].reshape(N_GROUPS, G)     # [512, 16]

    iota = np.ascontiguousarray(
        np.tile(np.tile(np.arange(G, dtype=ml_dtypes.bfloat16), ST)[None, :],
                (128, 1))
    )                                                      # [128, 512]
    consts = {
        "iota": iota,
        "idf": np.eye(G, dtype=np.float32),
        "wt": np.ascontiguousarray(W.T.astype(np.float32)),
        "bb": np.ascontiguousarray(b.astype(np.float32).reshape(1, D)),
        "ones": np.ones((1, D), dtype=np.float32),
    }

    in_maps = []
    for c in range(N_CORES):
        r0, r1 = c * CORE_ROWS, (c + 1) * CORE_ROWS
        g0, g1 = c * GROUPS_PER_CORE, (c + 1) * GROUPS_PER_CORE
        invc_core = invc_packed[g0:g1]                     # [64, 16]
        m = {
            "xg": np.ascontiguousarray(xg_grp[g0 * 128:g1 * 128]),
            "xt": np.ascontiguousarray(xt_pad[:, r0:r1]),
            # slot[p, g*ST+st] = slotvec[g*GROUP_ROWS + st*128 + p] (core-local)
            "slot": np.ascontiguousarray(
                slotvec[r0:r1].reshape(-1, 128).T.astype(ml_dtypes.bfloat16)
            ),
            # invcb[p, g*16+s] = invc at (g, s), replicated over partitions
            "invcb": np.ascontiguousarray(
                np.tile(invc_core.reshape(1, -1), (128, 1))
                .astype(ml_dtypes.bfloat16)
            ),
        }
        m.update(consts)
        in_maps.append(m)

    return in_maps, seg_ids


# --------------------------------------------------------------------------
# device program
# --------------------------------------------------------------------------

def _build_program(groups=GROUPS_PER_CORE):
    _patch_tile_tail()
    nc = bass.Bass("TRN2", debug=False)

    xg_h = nc.dram_tensor("xg", [groups * 128, ST * D], BF16, kind="ExternalInput")
    xt_h = nc.dram_tensor("xt", [128, groups * GROUP_ROWS], F8, kind="ExternalInput")
    slot_h = nc.dram_tensor("slot", [128, groups * ST], BF16, kind="ExternalInput")
    invcb_h = nc.dram_tensor("invcb", [128, groups * G], BF16, kind="ExternalInput")
    iota_h = nc.dram_tensor("iota", [128, ST * G], BF16, kind="ExternalInput")
    idf_h = nc.dram_tensor("idf", [G, G], F32, kind="ExternalInput")
    wt_h = nc.dram_tensor("wt", [D, D], F32, kind="ExternalInput")
    bb_h = nc.dram_tensor("bb", [1, D], F32, kind="ExternalInput")
    ones_h = nc.dram_tensor("ones", [1, D], F32, kind="ExternalInput")
    out_h = nc.dram_tensor("out", [groups * G, D], F32, kind="ExternalOutput")

    mult = mybir.AluOpType.mult
    is_eq = mybir.AluOpType.is_equal

    with tile.TileContext(nc) as tc:
        from contextlib import ExitStack
        with ExitStack() as ctx:
            cpool = ctx.enter_context(tc.tile_pool(name="consts", bufs=1))
            # tiny tensors needed by the very first DVE op go FIRST on the
            # scalar ring; everything else is deferred until after the first
            # two x pair transfers are queued (emitted in the main loop).
            slot_t = cpool.tile([128, groups * ST], BF16)
            nc.scalar.dma_start(out=slot_t[:], in_=slot_h.ap()[:])
            iota_t = cpool.tile([128, ST * G], BF16)
            nc.scalar.dma_start(out=iota_t[:], in_=iota_h.ap()[:])
            invcb_t = cpool.tile([128, groups * G], BF16)
            idf_t = cpool.tile([G, G], F32)
            wt_t = cpool.tile([D, D], F32)
            bb_t = cpool.tile([1, D], F32)
            ones_t = cpool.tile([1, D], F32)

            def emit_const_dmas():
                nc.sync.dma_start(out=invcb_t[:], in_=invcb_h.ap()[:])
                nc.sync.dma_start(out=idf_t[:], in_=idf_h.ap()[:])
                nc.sync.dma_start(out=wt_t[:], in_=wt_h.ap()[:])
                nc.sync.dma_start(out=bb_t[:], in_=bb_h.ap()[:])
                nc.sync.dma_start(out=ones_t[:], in_=ones_h.ap()[:])

            pooledT = cpool.tile([128, groups * G], F32)  # persistent result

            xgpool = ctx.enter_context(tc.tile_pool(name="xg", bufs=6))
            xtpool = ctx.enter_context(tc.tile_pool(name="xt", bufs=6))
            p8pool = ctx.enter_context(tc.tile_pool(name="p8", bufs=3))
            qpool = ctx.enter_context(tc.tile_pool(name="q", bufs=3))
            ctpool = ctx.enter_context(tc.tile_pool(name="ct", bufs=3))
            pgpool = ctx.enter_context(tc.tile_pool(name="pg", bufs=2))
            obpool = ctx.enter_context(tc.tile_pool(name="ob", bufs=2))

            with ExitStack() as psctx:
                psat_pool = psctx.enter_context(
                    tc.tile_pool(name="psAT", bufs=2, space="PSUM"))
                psaq_pool = psctx.enter_context(
                    tc.tile_pool(name="psAq", bufs=2, space="PSUM"))
                psp_pool = psctx.enter_context(
                    tc.tile_pool(name="psPool", bufs=2, space="PSUM"))
                pst_pool = psctx.enter_context(
                    tc.tile_pool(name="psT", bufs=1, space="PSUM"))
                pso_pool = psctx.enter_context(
                    tc.tile_pool(name="psO", bufs=1, space="PSUM"))

                stateA = {}      # per-group tiles handed from pass A to B1
                stateB = {}      # per-group tiles handed from B1 to B2
                pair_state = {}  # per-pair xg2/xt2 tiles

                def emit_dma_pair(i):
                    ring_a = nc.sync if i % 2 == 0 else nc.scalar
                    ring_b = nc.scalar if i % 2 == 0 else nc.sync
                    xg2 = xgpool.tile([128, 2 * ST * D], BF16)
                    if i == 0:
                        # same-ring split so the first A pass only waits 1 MB
                        ring_a.dma_start(
                            out=xg2[:, :ST * D], in_=xg_h.ap()[0:128, :])
                        ring_a.dma_start(
                            out=xg2[:, ST * D:], in_=xg_h.ap()[128:256, :])
                    else:
                        ring_a.dma_start(
                            out=xg2[:].rearrange("p (two f) -> p two f", two=2),
                            in_=xg_h.ap()[2 * i * 128:(2 * i + 2) * 128, :]
                                .rearrange("(two p) f -> p two f", two=2),
                        )
                    xt2 = xtpool.tile([128, 2 * GROUP_ROWS], F8)
                    ring_b.dma_start(
                        out=xt2[:],
                        in_=xt_h.ap()[:, 2 * i * GROUP_ROWS:
                                      (2 * i + 2) * GROUP_ROWS],
                    )
                    pair_state[i] = (xg2, xt2)

                def emit_passA(g):
                    xg2, xt2 = pair_state[g // 2]
                    xo = (g % 2) * ST * D
                    # one-hot P for the whole group in one DVE op
                    p8 = p8pool.tile([128, ST * G], BF16, tag="p")
                    nc.vector.tensor_tensor(
                        out=p8[:],
                        in0=slot_t[:, g * ST:(g + 1) * ST]
                            .to_broadcast([128, ST, G]),
                        in1=iota_t[:],
                        op=is_eq,
                    )
                    # segment sums, transposed: psAT[d, slot]
                    psAT = psat_pool.tile([128, G], F32, space="PSUM")
                    for st in range(ST):
                        nc.tensor.matmul(
                            out=psAT[:],
                            lhsT=xg2[:, xo + st * D:xo + (st + 1) * D],
                            rhs=p8[:, st * G:(st + 1) * G],
                            start=(st == 0),
                            stop=(st == ST - 1),
                        )
                    ct = ctpool.tile([128, G], BF16, tag="ct")
                    nc.vector.tensor_tensor(
                        out=ct[:], in0=psAT[:],
                        in1=invcb_t[:, g * G:(g + 1) * G], op=mult,
                    )
                    stateA[g] = (p8, ct)

                # qq mask is split into chunks so pass B2's first matmuls can
                # start before the whole group's mask is done
                QCH = 4
                CHST = ST // QCH + (ST % QCH > 0)  # subtiles per chunk

                def emit_passB1(g):
                    p8, ct = stateA.pop(g)
                    xg2, xt2 = pair_state[g // 2]
                    to = (g % 2) * GROUP_ROWS
                    # att: psAq[row, st*16+slot] = <x_row, c_slot>
                    psAq = psaq_pool.tile([128, ST * G], F32, space="PSUM")
                    for st in range(ST):
                        nc.tensor.matmul(
                            out=psAq[:, st * G:(st + 1) * G],
                            lhsT=xt2[:, to + st * 128:to + (st + 1) * 128],
                            rhs=ct[:],
                            start=True, stop=True,
                        )
                    qq = qpool.tile([128, ST * G], BF16, tag="q")
                    for c0 in range(0, ST, CHST):
                        c1 = min(c0 + CHST, ST)
                        nc.vector.tensor_tensor(
                            out=qq[:, c0 * G:c1 * G],
                            in0=psAq[:, c0 * G:c1 * G],
                            in1=p8[:, c0 * G:c1 * G], op=mult,
                        )
                    stateB[g] = qq

                def emit_window(w0):
                    # out[w0:w0+128, :] = pooled @ W^T + b
                    pso = pso_pool.tile([128, D], F32, space="PSUM")
                    nc.tensor.matmul(
                        out=pso[:], lhsT=pooledT[:, w0:w0 + 128],
                        rhs=wt_t[:], start=True, stop=False,
                    )
                    nc.tensor.matmul(
                        out=pso[:], lhsT=ones_t[:],
                        rhs=bb_t[:], start=False, stop=True,
                    )
                    ob = obpool.tile([128, D], F32)
                    nc.vector.tensor_copy(out=ob[:], in_=pso[:])
                    # keep the two HWDGE rings free for x transfers
                    nc.gpsimd.dma_start(
                        out=out_h.ap()[w0:w0 + 128, :], in_=ob[:],
                    )

                def emit_passB2(g):
                    qq = stateB.pop(g)
                    xg2, _ = pair_state[g // 2]
                    xo = (g % 2) * ST * D
                    psPool = psp_pool.tile([G, D], F32, space="PSUM")
                    for st in range(ST):
                        nc.tensor.matmul(
                            out=psPool[:],
                            lhsT=qq[:, st * G:(st + 1) * G],
                            rhs=xg2[:, xo + st * D:xo + (st + 1) * D],
                            start=(st == 0),
                            stop=(st == ST - 1),
                        )
                    if g % 2 == 1:
                        pair_state.pop(g // 2)
                    pg = pgpool.tile([G, D], F32)
                    nc.vector.tensor_copy(out=pg[:], in_=psPool[:])
                    psT = pst_pool.tile([128, G], F32, space="PSUM")
                    nc.tensor.transpose(out=psT[:], in_=pg[:], identity=idf_t[:])
                    nc.vector.tensor_tensor(
                        out=pooledT[:, g * G:(g + 1) * G], in0=psT[:],
                        in1=invcb_t[:, g * G:(g + 1) * G], op=mult,
                    )
                    if (g + 1) % 8 == 0:
                        emit_window((g + 1 - 8) * G)

                emit_dma_pair(0)
                emit_const_dmas()
                for i in range(groups // 2):
                    if i >= 1:
                        emit_dma_pair(i)
                    emit_passA(2 * i)
                    if i >= 1:
                        emit_passB1(2 * i - 1)
                        emit_passB2(2 * i - 1)
                    emit_passA(2 * i + 1)
                    emit_passB1(2 * i)
                    emit_passB2(2 * i)
                emit_passB1(groups - 1)
                emit_passB2(groups - 1)

    return nc


# --------------------------------------------------------------------------
# entry point
# --------------------------------------------------------------------------

def kernel(x, batch, W, b, num_segments):
    global LAST_RESULT
    x = np.asarray(x)
    batch = np.asarray(batch)
    W = np.asarray(W, dtype=np.float32)
    b = np.asarray(b, dtype=np.float32)

    in_maps, seg_ids = _host_prepare(x, batch, W, b)
    nc = _build_program()

    if os.environ.get("KERNEL_LDW_OPT", "0") == "1":
        # the baked compiler bundle disables the LDWEIGHTS fast path
        # (--enable-ldw-opt=false); our kernel is weight-load bound, so
        # re-enable it for this compile.
        from concourse.compiler_utils import get_compiler_flags, set_compiler_flags
        set_compiler_flags([
            f.replace("--enable-ldw-opt=false", "--enable-ldw-opt=true")
            for f in get_compiler_flags()
        ])

    trace = bool(int(os.environ.get("KERNEL_TRACE", "0")))
    res = run_bass_kernel_spmd(
        nc, in_maps, core_ids=list(range(N_CORES)),
        trace=trace, trace_cores=[0] if trace else None,
    )
    LAST_RESULT = res

    out = np.empty((B_SEGS, D), dtype=np.float32)
    packed = np.concatenate([res.results[c]["out"] for c in range(N_CORES)], axis=0)
    out[seg_ids.reshape(-1)] = packed
    return out
